# revision 2
# baseline (speedup 1.0000x reference)
"""Causal single-head attention on 8 Trainium2 NeuronCores (Bass/Tile), v2.

Problem: x[4,2048,1024] fp32, Wq/Wk/Wv[1024,1024];
  q,k,v = x@W.T ; S = q@k.T/sqrt(d) ; causal softmax ; out = P@v.

Sharding v2: core c -> batch b=c//2, fold-half h=c%2. The 16 query blocks
(128 rows each) of a batch are split between the two cores of the pair by
folded pairs (j, 15-j) so causal work balances exactly:
  h=0 -> blocks [0,3,4,7,8,11,12,15], h=1 -> [1,2,5,6,9,10,13,14].
Both orderings give the same per-slot key-chunk counts [1,1,2,2,3,3,4,4]
(block g needs keys [0,(g//4+1)*512) at 512 granularity), so one compiled
program is SPMD-uniform across cores; only tensor contents differ.

Each core projects Q for its own 1024 queries and the FULL K/V for its
batch (duplicated across the pair). This removes the inter-core collective
and the K/V DRAM round-trip entirely: ~2x more K/V projection matmuls,
but ~40MB/iter less HBM traffic and zero cross-core sync.

Causal skipping: scores/AV for block g only run over key chunks
[0, c*512), c = g//4+1 (37.5% less score/AV work than dense). Only the
diagonal chunk needs a mask; masks are host-precomputed additive tiles
(0 / -1e30) since the block index differs per core.

All matmul inputs are bf16 (PE streams 1 col/cycle, same as fp32r);
accumulation, softmax and the output stay fp32. End-to-end rel err ~5e-3.
"""

import sys

sys.path.insert(0, "/opt/trn_rl_repo")

from contextlib import ExitStack

import numpy as np
import ml_dtypes

import concourse.bass as bass
from concourse import bacc
import concourse.mybir as mybir
import concourse.tile as tile
from concourse.bass_utils import run_bass_kernel_spmd

F32 = mybir.dt.float32
BF16 = mybir.dt.bfloat16

B, N, D = 4, 2048, 1024
P = 128          # partition block
NQ = N // 2      # queries per core
ND = D // P      # 8 d-blocks
NO = D // P      # 8 o-blocks
NS = N // P      # 16 key blocks of 128
NKC = N // 512   # 4 key chunks of 512
MASK_VAL = -1.0e30
BLOCKS = {0: [0, 3, 4, 7, 8, 11, 12, 15], 1: [1, 2, 5, 6, 9, 10, 13, 14]}
CHUNKS = [1, 1, 2, 2, 3, 3, 4, 4]   # key-chunk count per slot (both halves)

_CACHE = {}


def _build_program(iters=1, phase="full"):
    nc = bacc.Bacc("TRN2", target_bir_lowering=False, debug=False, num_devices=8)
    xkvT = nc.dram_tensor("xkvT", [D, N], BF16, kind="ExternalInput").ap()
    xqT = nc.dram_tensor("xqT", [D, NQ], BF16, kind="ExternalInput").ap()
    wqT = nc.dram_tensor("wqT", [D, D], BF16, kind="ExternalInput").ap()
    wkT = nc.dram_tensor("wkT", [D, D], BF16, kind="ExternalInput").ap()
    wvT = nc.dram_tensor("wvT", [D, D], BF16, kind="ExternalInput").ap()
    masks = nc.dram_tensor("masks", [NQ, 512], F32, kind="ExternalInput").ap()
    ident_d = nc.dram_tensor("ident", [P, P], BF16, kind="ExternalInput").ap()
    out = nc.dram_tensor("out", [NQ, D], F32, kind="ExternalOutput").ap()

    with tile.TileContext(nc) as tc:
        if iters == 1:
            _attention_kernel(tc, out, xkvT, xqT, wqT, wkT, wvT, masks, ident_d)
        else:
            with tc.For_i(0, iters, 1):
                _attention_kernel(tc, out, xkvT, xqT, wqT, wkT, wvT, masks, ident_d)
    nc.compile()
    return nc


def _attention_kernel(tc, out, xkvT, xqT, wqT, wkT, wvT, masks, ident_d):
    nc = tc.nc

    with ExitStack() as ctx:
        const_pool = ctx.enter_context(tc.tile_pool(name="const", bufs=1))
        ident = const_pool.tile([P, P], BF16, tag="ident")
        nc.sync.dma_start(ident[:], ident_d[:, :])

        # ---- resident inputs ----
        xkv_pool = ctx.enter_context(tc.tile_pool(name="xkv", bufs=1))
        xkv = [xkv_pool.tile([P, N], BF16, tag=f"xkv{d}", name=f"xkv{d}") for d in range(ND)]
        for d in range(ND):
            nc.sync.dma_start(xkv[d][:], xkvT[d * P : (d + 1) * P, :])
        xq_pool = ctx.enter_context(tc.tile_pool(name="xq", bufs=1))
        xq = [xq_pool.tile([P, NQ], BF16, tag=f"xq{d}", name=f"xq{d}") for d in range(ND)]
        for d in range(ND):
            nc.sync.dma_start(xq[d][:], xqT[d * P : (d + 1) * P, :])

        # weight pool: wv reuses wk's slots (tag wA) after K-proj drains
        w_pool = ctx.enter_context(tc.tile_pool(name="w", bufs=1))

        # ---- resident projection outputs ----
        kt_pool = ctx.enter_context(tc.tile_pool(name="kt", bufs=1))
        KT = [kt_pool.tile([P, N], BF16, tag=f"kt{ob}", name=f"kt{ob}") for ob in range(NO)]
        qt_pool = ctx.enter_context(tc.tile_pool(name="qt", bufs=1))
        QT = [qt_pool.tile([P, NQ], BF16, tag=f"qt{ob}", name=f"qt{ob}") for ob in range(NO)]
        v_pool = ctx.enter_context(tc.tile_pool(name="v", bufs=1))
        V = [v_pool.tile([P, D], BF16, tag=f"v{sb}", name=f"v{sb}") for sb in range(NS)]

        # ================= projections =================
        with ExitStack() as pctx:
            psum_p = pctx.enter_context(
                tc.tile_pool(name="psum_p", bufs=4, space="PSUM")
            )

            # --- K projection: KT[o, k] for ALL 2048 keys ---
            wk = [w_pool.tile([P, D], BF16, tag=f"wA{d}", name=f"wk{d}") for d in range(ND)]
            for d in range(ND):
                nc.sync.dma_start(wk[d][:], wkT[d * P : (d + 1) * P, :])
            for ob in range(NO):
                kps = [psum_p.tile([P, 512], F32, tag="psp", name=f"kps{i}") for i in range(NKC)]
                for d in range(ND):
                    for kc in range(NKC):  # share the stationary wk slice
                        nc.tensor.matmul(
                            kps[kc][:],
                            wk[d][:, ob * P : (ob + 1) * P],
                            xkv[d][:, kc * 512 : (kc + 1) * 512],
                            start=(d == 0),
                            stop=(d == ND - 1),
                        )
                for kc in range(NKC):
                    nc.scalar.copy(
                        KT[ob][:, kc * 512 : (kc + 1) * 512], kps[kc][:]
                    )

            # --- Q projection (own 1024 queries, slot order) ---
            wq = [w_pool.tile([P, D], BF16, tag=f"wB{d}", name=f"wq{d}") for d in range(ND)]
            for d in range(ND):
                nc.sync.dma_start(wq[d][:], wqT[d * P : (d + 1) * P, :])
            for ob in range(NO):
                qps = [psum_p.tile([P, 512], F32, tag="psp", name=f"qps{i}") for i in range(2)]
                for d in range(ND):
                    for qc in range(2):
                        nc.tensor.matmul(
                            qps[qc][:],
                            wq[d][:, ob * P : (ob + 1) * P],
                            xq[d][:, qc * 512 : (qc + 1) * 512],
                            start=(d == 0),
                            stop=(d == ND - 1),
                        )
                for qc in range(2):
                    nc.vector.tensor_copy(
                        QT[ob][:, qc * 512 : (qc + 1) * 512], qps[qc][:]
                    )

            # --- V projection: V[k, o] for ALL 2048 keys ---
            wv = [w_pool.tile([P, D], BF16, tag=f"wA{d}", name=f"wv{d}") for d in range(ND)]
            for d in range(ND):
                nc.sync.dma_start(wv[d][:], wvT[d * P : (d + 1) * P, :])
            for sb in range(NS):
                vps = [psum_p.tile([P, 512], F32, tag="psp", name=f"vps{i}") for i in range(2)]
                for d in range(ND):
                    for oc in range(2):  # share the stationary xkv slice
                        nc.tensor.matmul(
                            vps[oc][:],
                            xkv[d][:, sb * P : (sb + 1) * P],
                            wv[d][:, oc * 512 : (oc + 1) * 512],
                            start=(d == 0),
                            stop=(d == ND - 1),
                        )
                for oc in range(2):
                    nc.scalar.copy(
                        V[sb][:, oc * 512 : (oc + 1) * 512], vps[oc][:]
                    )

        # ================= attention (folded causal slots) =================
        with ExitStack() as actx:
            s_pool = actx.enter_context(tc.tile_pool(name="s", bufs=2))
            p_pool = actx.enter_context(tc.tile_pool(name="p", bufs=2))
            mask_pool = actx.enter_context(tc.tile_pool(name="mask", bufs=2))
            stat_pool = actx.enter_context(tc.tile_pool(name="stat", bufs=4))
            pt_pool = actx.enter_context(tc.tile_pool(name="pt", bufs=2))
            o_pool = actx.enter_context(tc.tile_pool(name="o", bufs=2))
            psum_s = actx.enter_context(tc.tile_pool(name="psum_s", bufs=4, space="PSUM"))
            psum_t = actx.enter_context(tc.tile_pool(name="psum_t", bufs=2, space="PSUM"))
            psum_o = actx.enter_context(tc.tile_pool(name="psum_o", bufs=2, space="PSUM"))

            for s in range(8):
                c = CHUNKS[s]           # key chunks for this slot
                nk = c * 512            # keys covered
                M = mask_pool.tile([P, 512], F32, tag="mask")
                nc.sync.dma_start(M[:], masks[s * P : (s + 1) * P, :])

                S = s_pool.tile([P, N], F32, tag="s")
                sps = [psum_s.tile([P, 512], F32, tag="pss", name=f"sps{i}") for i in range(c)]
                for ob in range(NO):
                    for kc in range(c):  # share the stationary QT slice
                        nc.tensor.matmul(
                            sps[kc][:],
                            QT[ob][:, s * P : (s + 1) * P],
                            KT[ob][:, kc * 512 : (kc + 1) * 512],
                            start=(ob == 0),
                            stop=(ob == NO - 1),
                        )
                # evacuate: plain copy below the diagonal, mask-add on it
                for kc in range(c - 1):
                    nc.scalar.copy(S[:, kc * 512 : (kc + 1) * 512], sps[kc][:])
                nc.vector.tensor_tensor(
                    S[:, (c - 1) * 512 : c * 512],
                    sps[c - 1][:],
                    M[:],
                    mybir.AluOpType.add,
                )

                neg_max = stat_pool.tile([P, 1], F32, tag="negmax")
                nc.vector.reduce_max(
                    neg_max[:], S[:, 0:nk], axis=mybir.AxisListType.X, negate=True
                )
                zrow = stat_pool.tile([P, 1], F32, tag="zrow")
                Pt = p_pool.tile([P, N], BF16, tag="p")
                nc.scalar.activation(
                    Pt[:, 0:nk],
                    S[:, 0:nk],
                    mybir.ActivationFunctionType.Exp,
                    bias=neg_max[:],
                    scale=1.0,
                    accum_out=zrow[:],
                )
                rz = stat_pool.tile([P, 1], F32, tag="rz")
                nc.vector.reciprocal(rz[:], zrow[:])

                # AV over the covered key blocks
                op0 = psum_o.tile([P, 512], F32, tag="pso", name="op0")
                op1 = psum_o.tile([P, 512], F32, tag="pso", name="op1")
                nkb = 4 * c
                for kb in range(nkb):
                    tp = psum_t.tile([P, P], BF16, tag="pst")
                    nc.tensor.transpose(
                        tp[:], Pt[:, kb * P : (kb + 1) * P], ident[:]
                    )
                    pt = pt_pool.tile([P, P], BF16, tag="pt")
                    nc.vector.tensor_copy(pt[:], tp[:])
                    for oc, op in ((0, op0), (1, op1)):
                        nc.tensor.matmul(
                            op[:],
                            pt[:],
                            V[kb][:, oc * 512 : (oc + 1) * 512],
                            start=(kb == 0),
                            stop=(kb == nkb - 1),
                        )
                O = o_pool.tile([P, D], F32, tag="o")
                nc.vector.tensor_scalar_mul(O[:, 0:512], op0[:], rz[:])
                nc.vector.tensor_scalar_mul(O[:, 512:1024], op1[:], rz[:])
                nc.sync.dma_start(out[s * P : (s + 1) * P, :], O[:])


def _get_program(iters=1, phase="full"):
    key = ("nc", iters)
    if key not in _CACHE:
        _CACHE[key] = _build_program(iters, phase)
    return _CACHE[key]


def _host_prep(x, Wq, Wk, Wv):
    scale = np.float32(1.0 / np.sqrt(np.float32(D)))
    bf = ml_dtypes.bfloat16
    wqT = np.ascontiguousarray((np.asarray(Wq, np.float32) * scale).T.astype(bf))
    wkT = np.ascontiguousarray(np.asarray(Wk, np.float32).T.astype(bf))
    wvT = np.ascontiguousarray(np.asarray(Wv, np.float32).T.astype(bf))
    ident = np.eye(P, dtype=bf)

    # additive diagonal-chunk masks: rows g*128+p, cols j of chunk g//4
    # visible iff j <= (g%4)*128 + p
    jj = np.arange(512)[None, :]
    pp = np.arange(P)[:, None]
    mask_h = {}
    for h in (0, 1):
        ms = []
        for s2 in range(8):
            g = BLOCKS[h][s2]
            ms.append(
                np.where(jj <= (g % 4) * P + pp, 0.0, MASK_VAL).astype(np.float32)
            )
        mask_h[h] = np.concatenate(ms, axis=0)

    in_maps = []
    for core in range(8):
        b, h = core // 2, core % 2
        xb = np.asarray(x[b], np.float32)
        xrows = np.concatenate(
            [xb[g * P : (g + 1) * P] for g in BLOCKS[h]], axis=0
        )
        in_maps.append(
            {
                "xkvT": np.ascontiguousarray(xb.T.astype(bf)),
                "xqT": np.ascontiguousarray(xrows.T.astype(bf)),
                "wqT": wqT,
                "wkT": wkT,
                "wvT": wvT,
                "masks": mask_h[h],
                "ident": ident,
            }
        )
    return in_maps


def kernel(x, Wq, Wk, Wv):
    nc = _get_program()
    in_maps = _host_prep(x, Wq, Wk, Wv)
    res = run_bass_kernel_spmd(nc, in_maps, list(range(8)))
    _CACHE["last_results"] = res
    out = np.empty((B, N, D), np.float32)
    for core in range(8):
        b, h = core // 2, core % 2
        r = res.results[core]["out"]
        for s2, g in enumerate(BLOCKS[h]):
            out[b, g * P : (g + 1) * P] = r[s2 * P : (s2 + 1) * P]
    return out


# revision 3
# speedup vs baseline: 2.0351x; 2.0351x over previous
"""Causal single-head attention on 8 Trainium2 NeuronCores (Bass/Tile), v6.

Problem: x[4,2048,1024] fp32, Wq/Wk/Wv[1024,1024];
  q,k,v = x@W.T ; S = q@k.T/sqrt(d) ; causal softmax ; out = P@v.

Sharding v2: core c -> batch b=c//2, fold-half h=c%2. The 16 query blocks
(128 rows each) of a batch are split between the two cores of the pair by
folded pairs (j, 15-j) so causal work balances exactly:
  h=0 -> blocks [0,3,4,7,8,11,12,15], h=1 -> [1,2,5,6,9,10,13,14].
Both orderings give the same per-slot key-chunk counts [1,1,2,2,3,3,4,4]
(block g needs keys [0,(g//4+1)*512) at 512 granularity), so one compiled
program is SPMD-uniform across cores; only tensor contents differ.

Each core projects Q for its own 1024 queries and the FULL K/V for its
batch (duplicated across the pair). This removes the inter-core collective
and the K/V DRAM round-trip entirely: ~2x more K/V projection matmuls,
but ~40MB/iter less HBM traffic and zero cross-core sync.

Causal skipping: scores/AV for block g only run over key chunks
[0, c*512), c = g//4+1 (37.5% less score/AV work than dense). Only the
diagonal chunk needs a mask; masks are host-precomputed additive tiles
(0 / -1e30) since the block index differs per core.

Score trick: scores = q k^T = x (scale Wq^T Wk) x^T. The host
precomputes A = scale*Wq^T Wk; the device projects QT' = (x A)^T once
(replacing the Q projection at identical cost) and streams the resident
x^T tiles directly as the score moving operand -- the entire K
projection, its PSUM evacuations, SBUF residency and weight DMA vanish.

All matmul inputs are bf16 (PE streams 1 col/cycle, same as fp32r);
accumulation, softmax and the output stay fp32. End-to-end rel err ~5e-3.
"""

import sys

sys.path.insert(0, "/opt/trn_rl_repo")

from contextlib import ExitStack

import numpy as np
import ml_dtypes

import concourse.bass as bass
from concourse import bacc
import concourse.mybir as mybir
import concourse.tile as tile
from concourse.bass_utils import run_bass_kernel_spmd

F32 = mybir.dt.float32
BF16 = mybir.dt.bfloat16

B, N, D = 4, 2048, 1024
P = 128          # partition block
NQ = N // 2      # queries per core
ND = D // P      # 8 d-blocks
NO = D // P      # 8 o-blocks
NS = N // P      # 16 key blocks of 128
NKC = N // 512   # 4 key chunks of 512
MASK_VAL = -1.0e30
BLOCKS = {0: [0, 3, 4, 7, 8, 11, 12, 15], 1: [1, 2, 5, 6, 9, 10, 13, 14]}
CHUNKS = [1, 1, 2, 2, 3, 3, 4, 4]   # key-chunk count per slot (both halves)

_CACHE = {}


def _build_program(iters=1, phase="full"):
    nc = bacc.Bacc("TRN2", target_bir_lowering=False, debug=False, num_devices=8)
    xkvT = nc.dram_tensor("xkvT", [D, N], BF16, kind="ExternalInput").ap()
    xqT = nc.dram_tensor("xqT", [D, NQ], BF16, kind="ExternalInput").ap()
    amat = nc.dram_tensor("amat", [D, D], BF16, kind="ExternalInput").ap()
    wvT = nc.dram_tensor("wvT", [D, D], BF16, kind="ExternalInput").ap()
    masks = nc.dram_tensor("masks", [NQ, 512], F32, kind="ExternalInput").ap()
    ident_d = nc.dram_tensor("ident", [P, P], BF16, kind="ExternalInput").ap()
    out = nc.dram_tensor("out", [NQ, D], F32, kind="ExternalOutput").ap()

    with tile.TileContext(nc) as tc:
        # constants load once, outside the timing loop: a per-iteration ident
        # reload would block the SP DMA queue on the previous iteration's last
        # transpose (WAR), serializing all input DMAs to the iteration boundary
        const_pool = tc.alloc_tile_pool(name="const", bufs=1)
        ident = const_pool.tile([P, P], BF16, tag="ident")
        nc.sync.dma_start(ident[:], ident_d[:, :])
        if iters == 1:
            _attention_kernel(tc, out, xkvT, xqT, amat, wvT, masks, ident)
        else:
            with tc.For_i(0, iters, 1):
                _attention_kernel(tc, out, xkvT, xqT, amat, wvT, masks, ident)
        const_pool.release()
    nc.compile()
    return nc


def _attention_kernel(tc, out, xkvT, xqT, amat, wvT, masks, ident):
    nc = tc.nc

    with ExitStack() as ctx:
        # ---- resident inputs ----
        xkv_pool = ctx.enter_context(tc.tile_pool(name="xkv", bufs=1))
        xkv = [xkv_pool.tile([P, N], BF16, tag=f"xkv{d}", name=f"xkv{d}") for d in range(ND)]
        for d in range(ND):
            nc.sync.dma_start(xkv[d][:], xkvT[d * P : (d + 1) * P, :])
        xq_pool = ctx.enter_context(tc.tile_pool(name="xq", bufs=1))
        xq = [xq_pool.tile([P, NQ], BF16, tag=f"xq{d}", name=f"xq{d}") for d in range(ND)]
        for d in range(ND):
            nc.sync.dma_start(xq[d][:], xqT[d * P : (d + 1) * P, :])

        # weight pool: wv reuses wk's slots (tag wA) after K-proj drains
        w_pool = ctx.enter_context(tc.tile_pool(name="w", bufs=1))

        # ---- resident projection outputs ----
        qt_pool = ctx.enter_context(tc.tile_pool(name="qt", bufs=1))
        QT = [qt_pool.tile([P, NQ], BF16, tag=f"qt{ob}", name=f"qt{ob}") for ob in range(NO)]
        v_pool = ctx.enter_context(tc.tile_pool(name="v", bufs=1))
        V = [v_pool.tile([P, D], BF16, tag=f"v{sb}", name=f"v{sb}") for sb in range(NS)]

        # ================= projections =================
        with ExitStack() as pctx:
            psum_p = pctx.enter_context(
                tc.tile_pool(name="psum_p", bufs=4, space="PSUM")
            )

            # --- A projection: QT'[e, q] = (x A)^T for own 1024 queries ---
            wq = [w_pool.tile([P, D], BF16, tag=f"wB{d}", name=f"wq{d}") for d in range(ND)]
            for d in range(ND):
                nc.sync.dma_start(wq[d][:], amat[d * P : (d + 1) * P, :])
            for ob in range(NO):
                qps = [psum_p.tile([P, 512], F32, tag="psp", name=f"qps{i}") for i in range(2)]
                for d in range(ND):
                    for qc in range(2):
                        nc.tensor.matmul(
                            qps[qc][:],
                            wq[d][:, ob * P : (ob + 1) * P],
                            xq[d][:, qc * 512 : (qc + 1) * 512],
                            start=(d == 0),
                            stop=(d == ND - 1),
                        )
                for qc in range(2):
                    nc.vector.tensor_copy(
                        QT[ob][:, qc * 512 : (qc + 1) * 512], qps[qc][:]
                    )

            # --- V projection: V[k, o] for ALL 2048 keys ---
            wv = [w_pool.tile([P, D], BF16, tag=f"wA{d}", name=f"wv{d}") for d in range(ND)]
            for d in range(ND):
                nc.sync.dma_start(wv[d][:], wvT[d * P : (d + 1) * P, :])
            for sb in range(NS):
                vps = [psum_p.tile([P, 512], F32, tag="psp", name=f"vps{i}") for i in range(2)]
                for d in range(ND):
                    for oc in range(2):  # share the stationary xkv slice
                        nc.tensor.matmul(
                            vps[oc][:],
                            xkv[d][:, sb * P : (sb + 1) * P],
                            wv[d][:, oc * 512 : (oc + 1) * 512],
                            start=(d == 0),
                            stop=(d == ND - 1),
                        )
                for oc in range(2):
                    dst = V[sb][:, oc * 512 : (oc + 1) * 512]
                    if oc == 0:
                        nc.scalar.copy(dst, vps[oc][:])
                    else:
                        nc.vector.tensor_copy(dst, vps[oc][:])

        # ================= attention (folded causal slots) =================
        with ExitStack() as actx:
            s_pool = actx.enter_context(tc.tile_pool(name="s", bufs=2))
            p_pool = actx.enter_context(tc.tile_pool(name="p", bufs=2))
            mask_pool = actx.enter_context(tc.tile_pool(name="mask", bufs=2))
            stat_pool = actx.enter_context(tc.tile_pool(name="stat", bufs=4))
            pt_pool = actx.enter_context(tc.tile_pool(name="pt", bufs=3))
            o_pool = actx.enter_context(tc.tile_pool(name="o", bufs=2))
            psum_s = actx.enter_context(tc.tile_pool(name="psum_s", bufs=4, space="PSUM"))
            psum_t = actx.enter_context(tc.tile_pool(name="psum_t", bufs=2, space="PSUM"))
            psum_o = actx.enter_context(tc.tile_pool(name="psum_o", bufs=2, space="PSUM"))

            def emit_scores_softmax(s):
                c = CHUNKS[s]           # key chunks for this slot
                nk = c * 512            # keys covered
                M = mask_pool.tile([P, 512], F32, tag="mask")
                nc.sync.dma_start(M[:], masks[s * P : (s + 1) * P, :])

                S = s_pool.tile([P, N], F32, tag="s")
                sps = [psum_s.tile([P, 512], F32, tag="pss", name=f"sps{i}") for i in range(c)]
                for ob in range(NO):
                    for kc in range(c):  # share the stationary QT slice
                        nc.tensor.matmul(
                            sps[kc][:],
                            QT[ob][:, s * P : (s + 1) * P],
                            xkv[ob][:, kc * 512 : (kc + 1) * 512],
                            start=(ob == 0),
                            stop=(ob == NO - 1),
                        )
                # evacuate: plain copy below the diagonal, mask-add on it
                for kc in range(c - 1):
                    dst = S[:, kc * 512 : (kc + 1) * 512]
                    if kc % 2 == 0:
                        nc.scalar.copy(dst, sps[kc][:])
                    else:
                        nc.vector.tensor_copy(dst, sps[kc][:])
                nc.vector.tensor_tensor(
                    S[:, (c - 1) * 512 : c * 512],
                    sps[c - 1][:],
                    M[:],
                    mybir.AluOpType.add,
                )

                neg_max = stat_pool.tile([P, 1], F32, tag="negmax")
                nc.vector.reduce_max(
                    neg_max[:], S[:, 0:nk], axis=mybir.AxisListType.X, negate=True
                )
                zrow = stat_pool.tile([P, 1], F32, tag="zrow")
                Pt = p_pool.tile([P, N], BF16, tag="p")
                nc.scalar.activation(
                    Pt[:, 0:nk],
                    S[:, 0:nk],
                    mybir.ActivationFunctionType.Exp,
                    bias=neg_max[:],
                    scale=1.0,
                    accum_out=zrow[:],
                )
                rz = stat_pool.tile([P, 1], F32, tag="rz")
                nc.vector.reciprocal(rz[:], zrow[:])
                return (s, Pt, rz)

            def emit_av(state):
                s, Pt, rz = state
                # AV over the covered key blocks
                op0 = psum_o.tile([P, 512], F32, tag="pso", name="op0")
                op1 = psum_o.tile([P, 512], F32, tag="pso", name="op1")
                nkb = 2 * (s + 1)   # key blocks with nonzero P (fold property)
                for kb in range(nkb):
                    tp = psum_t.tile([P, P], BF16, tag="pst")
                    nc.tensor.transpose(
                        tp[:], Pt[:, kb * P : (kb + 1) * P], ident[:]
                    )
                    pt = pt_pool.tile([P, P], BF16, tag="pt")
                    nc.vector.tensor_copy(pt[:], tp[:])
                    for oc, op in ((0, op0), (1, op1)):
                        nc.tensor.matmul(
                            op[:],
                            pt[:],
                            V[kb][:, oc * 512 : (oc + 1) * 512],
                            start=(kb == 0),
                            stop=(kb == nkb - 1),
                        )
                O = o_pool.tile([P, D], F32, tag="o")
                nc.vector.tensor_scalar_mul(O[:, 0:512], op0[:], rz[:])
                nc.vector.tensor_scalar_mul(O[:, 512:1024], op1[:], rz[:])
                nc.sync.dma_start(out[s * P : (s + 1) * P, :], O[:])

            # software pipeline: scores(s+1) sit ahead of AV(s) in the PE
            # FIFO, so the PE streams them while softmax(s) runs on DVE/ACT
            prev = None
            for s in range(8):
                st = emit_scores_softmax(s)
                if prev is not None:
                    emit_av(prev)
                prev = st
            emit_av(prev)


def _get_program(iters=1, phase="full"):
    key = ("nc", iters)
    if key not in _CACHE:
        _CACHE[key] = _build_program(iters, phase)
    return _CACHE[key]


def _host_prep(x, Wq, Wk, Wv):
    scale = np.float32(1.0 / np.sqrt(np.float32(D)))
    bf = ml_dtypes.bfloat16
    amat_f = (np.asarray(Wq, np.float32).T @ np.asarray(Wk, np.float32)) * scale
    amat = np.ascontiguousarray(amat_f.astype(bf))
    wvT = np.ascontiguousarray(np.asarray(Wv, np.float32).T.astype(bf))
    ident = np.eye(P, dtype=bf)

    # additive diagonal-chunk masks: rows g*128+p, cols j of chunk g//4
    # visible iff j <= (g%4)*128 + p
    jj = np.arange(512)[None, :]
    pp = np.arange(P)[:, None]
    mask_h = {}
    for h in (0, 1):
        ms = []
        for s2 in range(8):
            g = BLOCKS[h][s2]
            ms.append(
                np.where(jj <= (g % 4) * P + pp, 0.0, MASK_VAL).astype(np.float32)
            )
        mask_h[h] = np.concatenate(ms, axis=0)

    in_maps = []
    for core in range(8):
        b, h = core // 2, core % 2
        xb = np.asarray(x[b], np.float32)
        xrows = np.concatenate(
            [xb[g * P : (g + 1) * P] for g in BLOCKS[h]], axis=0
        )
        in_maps.append(
            {
                "xkvT": np.ascontiguousarray(xb.T.astype(bf)),
                "xqT": np.ascontiguousarray(xrows.T.astype(bf)),
                "amat": amat,
                "wvT": wvT,
                "masks": mask_h[h],
                "ident": ident,
            }
        )
    return in_maps


def kernel(x, Wq, Wk, Wv):
    nc = _get_program()
    in_maps = _host_prep(x, Wq, Wk, Wv)
    res = run_bass_kernel_spmd(nc, in_maps, list(range(8)))
    _CACHE["last_results"] = res
    out = np.empty((B, N, D), np.float32)
    for core in range(8):
        b, h = core // 2, core % 2
        r = res.results[core]["out"]
        for s2, g in enumerate(BLOCKS[h]):
            out[b, g * P : (g + 1) * P] = r[s2 * P : (s2 + 1) * P]
    return out
